# revision 24
# baseline (speedup 1.0000x reference)
"""Trainium2 Bass kernel for the AgentLoss problem (raw bacc, manual sems).

Math: for each (l, b) the reference computes the masked cosine-similarity sum
    S = sum_{i != j} <x_i, x_j> / (|x_i| |x_j| + EPS)
over n=1024 agents with c=64 channels, then loss = sum_l mean_b S / (n(n-1)).

Since EPS (1e-5) is tiny vs |x_i||x_j| ~ 64, expand
    1/(m_i m_j + EPS) = r_i r_j - EPS r_i^2 r_j^2 + O(EPS^2),  r_i = 1/m_i
which makes the double sum separable:
    S ~= (|sum_i x_i r_i|^2 - sum_i msq_i r_i^2)
         - EPS * (|sum_i x_i r_i^2|^2 - sum_i msq_i r_i^4)
(order-1 truncation error ~3e-14 relative - validated vs fp64).

Device work per (l, b) pair: row norms (ACT square + DVE segmented reduce,
batched two pairs per op), r = ACT sqrt of DVE reciprocal, then thin fp32
matmuls contracting the agent axis with [r, r^2] weight columns, packed two
sub-tiles per matmul (N=128, half-garbage outputs that the host discards).
A run of warm-up matmuls keeps the PE clock at 2.4 GHz through the DMA/norm
phase, and a dummy sqrt up front pulls the ACT table load off the critical
path. Host does the final ~10k-flop combine in float64.

Sharding: data-parallel over batch b - core k takes b in {2k, 2k+1}, i.e.
8 (l, b_local) pairs per core. Each core returns a [4, 1152] block.
"""

from contextlib import ExitStack

import numpy as np

import concourse.bass as bass
from concourse import bacc, mybir
from concourse.bass_utils import run_bass_kernel_spmd

EPS = 1e-5
L, B, N, C = 4, 16, 1024, 64
P = 128            # SBUF partitions
T = N // P         # 8 agent sub-rows per partition
NCORES = 8
BPC = B // NCORES  # b per core
NPAIR = L * BPC    # (l, b_local) pairs per core
DP = 2             # pairs per square/reduce op
ND = NPAIR // DP   # dual-pair tiles
GP = 4             # pairs per norm/weights group
NG = NPAIR // GP   # 2 groups
GW = 8 * GP        # norm-group width (agents per partition per group)
NDUMMY = 16        # PE warm-up matmuls

F32 = mybir.dt.float32
OUT_W = NPAIR * P + NG * 2 * GW  # 1024 + 128


def build_nc() -> bass.Bass:
    nc = bacc.Bacc("TRN2", target_bir_lowering=False, debug=False, num_devices=NCORES)
    x = nc.declare_dram_parameter("x", [NPAIR, N, C], F32, isOutput=False)
    out = nc.declare_dram_parameter("out", [4, OUT_W], F32, isOutput=True)

    ctx = ExitStack()
    with ctx:
        def sb(name, shape):
            return ctx.enter_context(nc.sbuf_tensor(name, shape, F32))

        xd = [sb(f"xd{d}", [P, DP * T, C]) for d in range(ND)]
        xsq = [sb(f"xsq{d}", [P, DP * T, C]) for d in range(ND)]
        msq = [sb(f"msq{g}", [P, GW]) for g in range(NG)]
        inv = [sb(f"inv{g}", [P, GW]) for g in range(NG)]
        RR = [sb(f"RR{g}", [P, 2 * GW]) for g in range(NG)]
        PQ = [sb(f"PQ{g}", [P, 2 * GW]) for g in range(NG)]
        ones = sb("ones", [P, 1])
        zsc = sb("zsc", [P, 512])
        scr_s = sb("scr_s", [P, 1])
        stage = sb("stage", [4, OUT_W])
        psum_s = ctx.enter_context(nc.psum_tensor("psum_s", [4, NPAIR * P], F32))
        psum_pq = ctx.enter_context(nc.psum_tensor("psum_pq", [1, NG * 2 * GW], F32))
        psum_scr = ctx.enter_context(nc.psum_tensor("psum_scr", [1, 512], F32))

        s_dma = [nc.alloc_semaphore(f"s_dma{j}") for j in range(NPAIR)]
        s_dmo = nc.alloc_semaphore("s_dmo")
        s_z = nc.alloc_semaphore("s_z")      # warm-up scratch ready
        s_act = nc.alloc_semaphore("s_act")  # squares done (per dual)
        s_inv = nc.alloc_semaphore("s_inv")  # reciprocal done (per group)
        s_acc = nc.alloc_semaphore("s_acc")  # ACT chain: sqrt -> r^2 square
        s_rr = nc.alloc_semaphore("s_rr")    # weights ready (per group)
        s_pq = nc.alloc_semaphore("s_pq")    # PQ ready (per group)
        s_pe = nc.alloc_semaphore("s_pe")    # matmul progress (1, 2, 3)
        s_stage = nc.alloc_semaphore("s_stage")
        s_dve = nc.alloc_semaphore("s_dve")  # DVE same-engine RAW chain
        s_gp = nc.alloc_semaphore("s_gp")    # GpSimd same-engine RAW chain
        sems = s_dma + [s_dmo, s_z, s_act, s_inv, s_acc, s_rr, s_pq, s_pe,
                        s_stage, s_dve, s_gp]

        def rv4(t, lo=None, hi=None):
            # view a [P, GW] (or slice of [P, 2*GW]) tile as (slot, tt, f)
            ap = t[:] if lo is None else t[:, lo:hi]
            return ap.rearrange("p (s tt f) -> p s tt f", s=GP, tt=4)

        with nc.Block() as block:

            @block.sync
            def _(sync):
                for j in range(NPAIR):
                    d, half = divmod(j, DP)
                    sync.dma_start(
                        out=xd[d][:, half * T : (half + 1) * T, :],
                        in_=x[j].rearrange("(p t) c -> p t c", p=P),
                    ).then_inc(s_dma[j], 16)
                sync.wait_ge(s_stage, 3)
                sync.dma_start(out=out[:], in_=stage[:]).then_inc(s_dmo, 16)
                sync.wait_ge(s_dmo, 16)

            @block.scalar
            def _(scalar):
                # dummy sqrt: loads the sqrt+square ACT table set during the
                # DMA phase instead of mid-pipeline
                scalar.sqrt(scr_s[:], zsc[:, 0:1])._wait_ge(s_z, 1)

                def sq_dual(d):
                    scalar.wait_ge(s_dma[2 * d], 16)
                    scalar.square(xsq[d][:], xd[d][:])._wait_ge(
                        s_dma[2 * d + 1], 16
                    ).then_inc(s_act)

                def norm_group(g):
                    # r into RR[(slot,tt,0:2)], r^2 into RR[(slot,tt,2:4)]
                    scalar.activation(
                        rv4(RR[g], 0, None)[:, :, :, 0:2],
                        rv4(inv[g]),
                        mybir.ActivationFunctionType.Sqrt,
                    )._wait_ge(s_inv, g + 1).then_inc(s_acc)
                    scalar.square(
                        rv4(RR[g], 0, None)[:, :, :, 2:4],
                        rv4(RR[g], 0, None)[:, :, :, 0:2],
                    )._wait_ge(s_acc, g + 1).then_inc(s_rr)

                sq_dual(0)
                sq_dual(1)
                sq_dual(2)
                norm_group(0)
                sq_dual(3)
                norm_group(1)
                scalar.copy(
                    stage[0:1, NPAIR * P : OUT_W], psum_pq[:]
                )._wait_ge(s_pe, 3).then_inc(s_stage)

            @block.vector
            def _(vector):
                vector.memset(stage[:, NPAIR * P : OUT_W], 0.0)
                vd = [0]
                for g in range(NG):
                    for dd in range(GP // DP):
                        d = g * (GP // DP) + dd
                        vector.tensor_reduce(
                            out=msq[g][:, dd * DP * T : (dd + 1) * DP * T],
                            in_=xsq[d][:],
                            axis=mybir.AxisListType.X,
                            op=mybir.AluOpType.add,
                        )._wait_ge(s_act, d + 1).then_inc(s_dve)
                        vd[0] += 1
                    vector.reciprocal(out=inv[g][:], in_=msq[g][:])._wait_ge(
                        s_dve, vd[0]
                    ).then_inc(s_inv)
                half = NPAIR * P // 2
                vector.tensor_copy(
                    stage[:, 0:half], psum_s[:, 0:half]
                )._wait_ge(s_pe, 1).then_inc(s_stage)
                vector.tensor_copy(
                    stage[:, half : NPAIR * P], psum_s[:, half : NPAIR * P]
                )._wait_ge(s_pe, 2).then_inc(s_stage)

            @block.gpsimd
            def _(gpsimd):
                gpsimd.memset(ones[:], 1.0)
                gpsimd.memset(zsc[:], 0.0).then_inc(s_z)
                for g in range(NG):
                    r2v = rv4(RR[g], 0, None)[:, :, :, 2:4]
                    gpsimd.tensor_mul(
                        rv4(PQ[g], 0, GW), rv4(msq[g]), r2v
                    )._wait_ge(s_rr, g + 1).then_inc(s_gp)
                    gpsimd.tensor_mul(
                        rv4(PQ[g], GW, 2 * GW), rv4(PQ[g], 0, GW), r2v
                    )._wait_ge(s_gp, g + 1).then_inc(s_pq)

            @block.tensor
            def _(tensor):
                # warm-up: keeps the PE HAM clock at 2.4 GHz until real work
                tensor.wait_ge(s_z, 1)
                for i in range(NDUMMY):
                    tensor.matmul(
                        psum_scr[:], ones[:], zsc[:],
                        start=(i == 0), stop=(i == NDUMMY - 1),
                    )
                for g in range(NG):
                    tensor.wait_ge(s_rr, g + 1)
                    for slot in range(GP):
                        j = g * GP + slot
                        d, half = divmod(j, DP)
                        tensor.wait_ge(s_dma[j], 16)
                        for tt in range(T // 2):
                            mm = tensor.matmul(
                                psum_s[:, P * j : P * (j + 1)],
                                RR[g][:, slot * 16 + tt * 4 : slot * 16 + tt * 4 + 4],
                                xd[d][:, half * T + 2 * tt : half * T + 2 * tt + 2, :],
                                start=(tt == 0),
                                stop=(tt == T // 2 - 1),
                            )
                            if slot == GP - 1 and tt == T // 2 - 1:
                                mm.then_inc(s_pe)
                    tensor.wait_ge(s_pq, g + 1)
                    mm = tensor.matmul(
                        psum_pq[:, 2 * GW * g : 2 * GW * (g + 1)],
                        ones[:],
                        PQ[g][:].rearrange("p (k s) -> p k s", k=2),
                        start=True,
                        stop=True,
                    )
                    if g == NG - 1:
                        mm.then_inc(s_pe)

        for s in sems:
            nc.sync.sem_clear(s)

    nc.compile()
    return nc


_NC_CACHE = None


def _get_nc():
    global _NC_CACHE
    if _NC_CACHE is None:
        _NC_CACHE = build_nc()
    return _NC_CACHE


def run_cores(x_full: np.ndarray, trace: bool = False):
    """Shard, run on 8 NeuronCores, return (per-core out blocks, results obj)."""
    nc = _get_nc()
    in_maps = []
    for k in range(NCORES):
        shard = np.ascontiguousarray(
            x_full[:, BPC * k : BPC * (k + 1)].reshape(NPAIR, N, C)
        )
        in_maps.append({"x": shard})
    res = run_bass_kernel_spmd(nc, in_maps, list(range(NCORES)), trace=trace)
    outs = [res.results[k]["out"] for k in range(NCORES)]
    return outs, res


def reduce_host(outs) -> np.ndarray:
    total = 0.0
    for blk in outs:
        blk = blk.astype(np.float64)
        for j in range(NPAIR):
            g, slot = divmod(j, GP)
            s = blk[0, P * j : P * j + 64] + blk[1, P * j + 64 : P * j + 128]
            s2 = blk[2, P * j : P * j + 64] + blk[3, P * j + 64 : P * j + 128]
            pqb = blk[0, NPAIR * P + 2 * GW * g : NPAIR * P + 2 * GW * (g + 1)]
            t_sum = pqb[slot * 8 : slot * 8 + 8].sum()
            t2_sum = pqb[GW + slot * 8 : GW + slot * 8 + 8].sum()
            S0 = np.dot(s, s) - t_sum
            S1 = np.dot(s2, s2) - t2_sum
            total += S0 - EPS * S1
    loss = total / (N * (N - 1)) / B
    return np.array(loss, dtype=np.float32)


def kernel(updated_agents: np.ndarray) -> np.ndarray:
    outs, _ = run_cores(np.asarray(updated_agents))
    return reduce_host(outs)


# revision 26
# speedup vs baseline: 1.4956x; 1.4956x over previous
"""Trainium2 Bass kernel for the AgentLoss problem (raw bacc, manual sems).

Math: for each (l, b) the reference computes the masked cosine-similarity sum
    S = sum_{i != j} <x_i, x_j> / (|x_i| |x_j| + EPS)
over n=1024 agents with c=64 channels, then loss = sum_l mean_b S / (n(n-1)).

Since EPS (1e-5) is tiny vs |x_i||x_j| ~ 64, expand
    1/(m_i m_j + EPS) = r_i r_j - EPS r_i^2 r_j^2 + O(EPS^2),  r_i = 1/m_i
which makes the double sum separable:
    S ~= (|sum_i x_i r_i|^2 - sum_i msq_i r_i^2)
         - EPS * (|sum_i x_i r_i^2|^2 - sum_i msq_i r_i^4)
(order-1 truncation error ~3e-14 relative - validated vs fp64).

Device work per (l, b) pair: row norms (square + segmented reduce, squares
split ACT/GpSimd to balance engines), r = ACT sqrt of DVE reciprocal, then
thin fp32 matmuls contracting the agent axis with [r, r^2] weight columns,
packed two sub-tiles per matmul (N=128, half-garbage outputs the host
discards). A dummy sqrt up front pulls the ACT table load off the critical
path. Host does the final ~10k-flop combine in float64.

Sharding: data-parallel over batch b - core k takes b in {2k, 2k+1}, i.e.
8 (l, b_local) pairs per core. Each core returns a [4, 1152] block.
"""

from contextlib import ExitStack

import numpy as np

import concourse.bass as bass
from concourse import bacc, mybir
from concourse.bass_utils import run_bass_kernel_spmd

EPS = 1e-5
L, B, N, C = 4, 16, 1024, 64
P = 128            # SBUF partitions
T = N // P         # 8 agent sub-rows per partition
NCORES = 8
BPC = B // NCORES  # b per core
NPAIR = L * BPC    # (l, b_local) pairs per core
GP = 2             # pairs per norm/weights group
NG = NPAIR // GP   # 4 groups
GW = 8 * GP        # norm-group width (agents per partition per group)
N_GP_SQ = 2        # trailing squares offloaded to GpSimd

F32 = mybir.dt.float32
OUT_W = NPAIR * P + NG * 2 * GW  # 1024 + 128


def build_nc() -> bass.Bass:
    nc = bacc.Bacc("TRN2", target_bir_lowering=False, debug=False, num_devices=NCORES)
    x = nc.declare_dram_parameter("x", [NPAIR, N, C], F32, isOutput=False)
    out = nc.declare_dram_parameter("out", [4, OUT_W], F32, isOutput=True)

    ctx = ExitStack()
    with ctx:
        def sb(name, shape):
            return ctx.enter_context(nc.sbuf_tensor(name, shape, F32))

        xp = [sb(f"xp{j}", [P, T, C]) for j in range(NPAIR)]
        xsq = [sb(f"xsq{j}", [P, T, C]) for j in range(NPAIR)]
        msq = [sb(f"msq{g}", [P, GW]) for g in range(NG)]
        inv = [sb(f"inv{g}", [P, GW]) for g in range(NG)]
        RR = [sb(f"RR{g}", [P, 2 * GW]) for g in range(NG)]
        PQ = [sb(f"PQ{g}", [P, 2 * GW]) for g in range(NG)]
        ones = sb("ones", [P, 1])
        scr_s = sb("scr_s", [P, 1])
        stage = sb("stage", [4, OUT_W])
        psum_s = ctx.enter_context(nc.psum_tensor("psum_s", [4, NPAIR * P], F32))
        psum_pq = ctx.enter_context(nc.psum_tensor("psum_pq", [1, NG * 2 * GW], F32))

        s_dma = [nc.alloc_semaphore(f"s_dma{j}") for j in range(NPAIR)]
        s_dmo = nc.alloc_semaphore("s_dmo")
        s_z = nc.alloc_semaphore("s_z")        # ones ready
        s_act = nc.alloc_semaphore("s_act")    # ACT squares done (ordered)
        s_actg = nc.alloc_semaphore("s_actg")  # GpSimd squares done (ordered)
        s_inv = nc.alloc_semaphore("s_inv")    # reciprocal done (per group)
        s_acc = nc.alloc_semaphore("s_acc")    # ACT chain: sqrt -> r^2 square
        s_rr = nc.alloc_semaphore("s_rr")      # weights ready (per group)
        s_pq = nc.alloc_semaphore("s_pq")      # PQ ready (per group)
        s_pe = nc.alloc_semaphore("s_pe")      # matmul progress (1..5)
        s_stage = nc.alloc_semaphore("s_stage")
        s_dve = nc.alloc_semaphore("s_dve")    # DVE same-engine RAW chain
        s_gp = nc.alloc_semaphore("s_gp")      # GpSimd same-engine RAW chain
        sems = s_dma + [s_dmo, s_z, s_act, s_actg, s_inv, s_acc, s_rr, s_pq,
                        s_pe, s_stage, s_dve, s_gp]

        N_ACT_SQ = NPAIR - N_GP_SQ

        def rv4(t, lo=None, hi=None):
            # view a [P, GW] tile (or half of a [P, 2*GW] tile) as (slot, tt, f)
            ap = t[:] if lo is None else t[:, lo:hi]
            return ap.rearrange("p (s tt f) -> p s tt f", s=GP, tt=4)

        with nc.Block() as block:

            @block.sync
            def _(sync):
                for j in range(NPAIR):
                    sync.dma_start(
                        out=xp[j][:], in_=x[j].rearrange("(p t) c -> p t c", p=P)
                    ).then_inc(s_dma[j], 16)
                sync.wait_ge(s_stage, 3)
                sync.dma_start(out=out[:], in_=stage[:]).then_inc(s_dmo, 16)
                sync.wait_ge(s_dmo, 16)

            @block.scalar
            def _(scalar):
                # dummy sqrt: pulls the sqrt+square ACT table load into the
                # DMA phase
                scalar.sqrt(scr_s[:], ones[:])._wait_ge(s_z, 1)

                def norm_group(g):
                    # r into RR[(slot,tt,0:2)], r^2 into RR[(slot,tt,2:4)]
                    scalar.activation(
                        rv4(RR[g], 0, None)[:, :, :, 0:2],
                        rv4(inv[g]),
                        mybir.ActivationFunctionType.Sqrt,
                    )._wait_ge(s_inv, g + 1).then_inc(s_acc)
                    scalar.square(
                        rv4(RR[g], 0, None)[:, :, :, 2:4],
                        rv4(RR[g], 0, None)[:, :, :, 0:2],
                    )._wait_ge(s_acc, g + 1).then_inc(s_rr)

                def sq(j):
                    scalar.square(xsq[j][:], xp[j][:])._wait_ge(
                        s_dma[j], 16
                    ).then_inc(s_act)

                sq(0)
                sq(1)
                sq(2)
                norm_group(0)
                sq(3)
                norm_group(1)
                sq(4)
                sq(5)
                norm_group(2)
                norm_group(3)
                scalar.copy(
                    stage[0:1, NPAIR * P : OUT_W], psum_pq[:]
                )._wait_ge(s_pe, 5).then_inc(s_stage)

            @block.vector
            def _(vector):
                vector.memset(stage[:, NPAIR * P : OUT_W], 0.0)
                vd = [0]
                for g in range(NG):
                    for slot in range(GP):
                        j = g * GP + slot
                        red = vector.tensor_reduce(
                            out=msq[g][:, slot * 8 : slot * 8 + 8],
                            in_=xsq[j][:],
                            axis=mybir.AxisListType.X,
                            op=mybir.AluOpType.add,
                        )
                        if j < N_ACT_SQ:
                            red._wait_ge(s_act, j + 1)
                        else:
                            red._wait_ge(s_actg, j - N_ACT_SQ + 1)
                        red.then_inc(s_dve)
                        vd[0] += 1
                    vector.reciprocal(out=inv[g][:], in_=msq[g][:])._wait_ge(
                        s_dve, vd[0]
                    ).then_inc(s_inv)
                half = NPAIR * P // 2
                vector.tensor_copy(
                    stage[:, 0:half], psum_s[:, 0:half]
                )._wait_ge(s_pe, 2).then_inc(s_stage)
                vector.tensor_copy(
                    stage[:, half : NPAIR * P], psum_s[:, half : NPAIR * P]
                )._wait_ge(s_pe, 4).then_inc(s_stage)

            @block.gpsimd
            def _(gpsimd):
                gpsimd.memset(ones[:], 1.0).then_inc(s_z)
                for jj in range(N_GP_SQ):
                    j = N_ACT_SQ + jj
                    gpsimd.tensor_mul(xsq[j][:], xp[j][:], xp[j][:])._wait_ge(
                        s_dma[j], 16
                    ).then_inc(s_actg)
                for g in range(NG):
                    r2v = rv4(RR[g], 0, None)[:, :, :, 2:4]
                    gpsimd.tensor_mul(
                        rv4(PQ[g], 0, GW), rv4(msq[g]), r2v
                    )._wait_ge(s_rr, g + 1).then_inc(s_gp)
                    gpsimd.tensor_mul(
                        rv4(PQ[g], GW, 2 * GW), rv4(PQ[g], 0, GW), r2v
                    )._wait_ge(s_gp, g + 1).then_inc(s_pq)

            @block.tensor
            def _(tensor):
                for g in range(NG):
                    tensor.wait_ge(s_rr, g + 1)
                    for slot in range(GP):
                        j = g * GP + slot
                        tensor.wait_ge(s_dma[j], 16)
                        for tt in range(T // 2):
                            mm = tensor.matmul(
                                psum_s[:, P * j : P * (j + 1)],
                                RR[g][:, slot * 16 + tt * 4 : slot * 16 + tt * 4 + 4],
                                xp[j][:, 2 * tt : 2 * tt + 2, :],
                                start=(tt == 0),
                                stop=(tt == T // 2 - 1),
                            )
                            if slot == GP - 1 and tt == T // 2 - 1:
                                mm.then_inc(s_pe)
                # pq matmuls at the stream end: the gpsimd chain that feeds
                # them is slow, and an in-order mid-stream wait would stall
                # the later s-matmul groups
                for g in range(NG):
                    tensor.wait_ge(s_pq, g + 1)
                    mm = tensor.matmul(
                        psum_pq[:, 2 * GW * g : 2 * GW * (g + 1)],
                        ones[:],
                        PQ[g][:].rearrange("p (k s) -> p k s", k=2),
                        start=True,
                        stop=True,
                    )
                    if g == NG - 1:
                        mm.then_inc(s_pe)

        for s in sems:
            nc.sync.sem_clear(s)

    nc.compile()
    return nc


_NC_CACHE = None


def _get_nc():
    global _NC_CACHE
    if _NC_CACHE is None:
        _NC_CACHE = build_nc()
    return _NC_CACHE


def run_cores(x_full: np.ndarray, trace: bool = False):
    """Shard, run on 8 NeuronCores, return (per-core out blocks, results obj)."""
    nc = _get_nc()
    in_maps = []
    for k in range(NCORES):
        shard = np.ascontiguousarray(
            x_full[:, BPC * k : BPC * (k + 1)].reshape(NPAIR, N, C)
        )
        in_maps.append({"x": shard})
    res = run_bass_kernel_spmd(nc, in_maps, list(range(NCORES)), trace=trace)
    outs = [res.results[k]["out"] for k in range(NCORES)]
    return outs, res


def reduce_host(outs) -> np.ndarray:
    total = 0.0
    for blk in outs:
        blk = blk.astype(np.float64)
        for j in range(NPAIR):
            g, slot = divmod(j, GP)
            s = blk[0, P * j : P * j + 64] + blk[1, P * j + 64 : P * j + 128]
            s2 = blk[2, P * j : P * j + 64] + blk[3, P * j + 64 : P * j + 128]
            pqb = blk[0, NPAIR * P + 2 * GW * g : NPAIR * P + 2 * GW * (g + 1)]
            t_sum = pqb[slot * 8 : slot * 8 + 8].sum()
            t2_sum = pqb[GW + slot * 8 : GW + slot * 8 + 8].sum()
            S0 = np.dot(s, s) - t_sum
            S1 = np.dot(s2, s2) - t2_sum
            total += S0 - EPS * S1
    loss = total / (N * (N - 1)) / B
    return np.array(loss, dtype=np.float32)


def kernel(updated_agents: np.ndarray) -> np.ndarray:
    outs, _ = run_cores(np.asarray(updated_agents))
    return reduce_host(outs)
